# revision 4
# baseline (speedup 1.0000x reference)
"""GAT layer (DiseaseGraphGAT) Trainium2 kernel, 8-way sharded over query rows.

Math (reference):
    s1 = emb @ attn[:D], s2 = emb @ attn[D:]          (N,)
    e  = leaky_relu(s1_i + s2_j, 0.2) masked by adj
    alpha = softmax(e, rows); out = alpha @ emb

Reformulation used here (per-row-scale invariant form; any positive per-i
factor cancels in the softmax ratio):
    w_ij / exp(s1_i) = exp(s2_j) * G_ij,  G_ij = exp(relu(-0.8*(s1_i+s2_j)))
    num_i = sum_j adj_ij * G_ij * E4_j      with E4 = diag(exp(s2)) @ emb
    Z_i   = sum_j adj_ij * G_ij * q4_j      with q4 = exp(s2)
    out_i = num_i / Z_i

Device pipeline per (128-row i-block, 2048-col j-strip), natural layout:
    1. DVE tensor_scalar:  r = max(S2B_scaled + bias_i, 0)       [f32]
       where S2B_scaled = -0.8*s2/256 broadcast, bias_i = -0.8*s1_i/256
    2. SWDGE accum-DMA:    r += cast_f32(adj)    (the HBM adj stream)
    3. ACT:                aw = Exp(256*r - 256) -> bf16
       adj=1 -> exp(relu(-0.8 x)) ; adj=0 -> exp(...-256) == 0  (exact mask)
    4. xbar DMA transpose: per-128-block transposed copies into AWT strip
    5. PE: psum_num[d,i] += E4_chunk.T @ AWT ; psum_z[0,i] += q4_chunk.T @ AWT

Host does the tiny O(N*D) precompute (s1, s2, E4) and the final divide.
"""

import sys

sys.path.insert(0, "/opt/trn_rl_repo")

import numpy as np
import ml_dtypes

import concourse.bacc as bacc
import concourse.mybir as mybir
import concourse.tile as tile
from concourse.bass_utils import run_bass_kernel_spmd

N = 8192
D = 128
NCORES = 8
NI_CORE = N // NCORES          # 1024 query rows per core
IBLK = 128                     # i-block (partition dim)
ICHUNK = 512                   # i extent per psum accumulation group
JSTRIP = 2048                  # j extent per build tile
NJC = N // 128                 # 64 j-chunks of 128
BIG = 256.0

_cache = {}


def _build_program(trace_friendly=False):
    if "nc" in _cache:
        return _cache["nc"]
    nc = bacc.Bacc("TRN2", target_bir_lowering=False, debug=False)
    adj_d = nc.declare_dram_parameter("adjs", [NI_CORE, N], mybir.dt.int32, isOutput=False)
    # packed preamble: cols [0:8]=per-i-block bias, col 8 = -BIG, cols 9: = -0.8*s2/BIG
    pre_d = nc.declare_dram_parameter("pre", [128, 9 + N], mybir.dt.float32, isOutput=False)
    e4_d = nc.declare_dram_parameter("e4", [128, NJC * D], mybir.dt.bfloat16, isOutput=False)
    q4_d = nc.declare_dram_parameter("q4", [128, NJC], mybir.dt.bfloat16, isOutput=False)
    numt_d = nc.declare_dram_parameter("numt", [D, NI_CORE], mybir.dt.float32, isOutput=True)
    z_d = nc.declare_dram_parameter("z", [1, NI_CORE], mybir.dt.float32, isOutput=True)

    NSTRIP = N // JSTRIP                # 4 j-strips
    JC_PER_STRIP = JSTRIP // 128        # 16 chunks per strip
    IB_PER_CHUNK = ICHUNK // IBLK       # 4 i-blocks per i-chunk
    NICHUNK = NI_CORE // ICHUNK         # 2 i-chunks per core

    with tile.TileContext(nc) as tc:
        with (
            tc.tile_pool(name="pre", bufs=1) as pre_pool,
            tc.tile_pool(name="work", bufs=3) as work,
            tc.tile_pool(name="awt", bufs=2) as awt_pool,
            tc.tile_pool(name="outp", bufs=2) as outp,
            tc.tile_pool(name="ps", bufs=2, space="PSUM") as ps,
        ):
            pre = pre_pool.tile([128, 9 + N], mybir.dt.float32)
            nc.sync.dma_start(out=pre[:], in_=pre_d[:])
            e4 = pre_pool.tile([128, NJC * D], mybir.dt.bfloat16)
            nc.sync.dma_start(out=e4[:], in_=e4_d[:])
            q4 = pre_pool.tile([128, NJC], mybir.dt.bfloat16)
            nc.sync.dma_start(out=q4[:], in_=q4_d[:])
            nbias = pre[:, 8:9]

            for ic in range(NICHUNK):
                ps_num = ps.tile([D, ICHUNK], mybir.dt.float32, tag="psnum")
                ps_z = ps.tile([1, ICHUNK], mybir.dt.float32, tag="psz")
                for js in range(NSTRIP):
                    awt = awt_pool.tile([128, JC_PER_STRIP * ICHUNK], mybir.dt.bfloat16)
                    for ib in range(IB_PER_CHUNK):
                        gib = ic * IB_PER_CHUNK + ib   # global i-block in core
                        r = work.tile([IBLK, JSTRIP], mybir.dt.float32, tag="r")
                        nc.vector.tensor_scalar(
                            r[:], pre[:, 9 + js * JSTRIP: 9 + (js + 1) * JSTRIP],
                            pre[:, gib:gib + 1], 0.0,
                            mybir.AluOpType.add, mybir.AluOpType.max)
                        nc.gpsimd.dma_start(
                            out=r[:],
                            in_=adj_d[gib * IBLK:(gib + 1) * IBLK,
                                      js * JSTRIP:(js + 1) * JSTRIP],
                            accum_op=mybir.AluOpType.add)
                        aw = work.tile([IBLK, JSTRIP], mybir.dt.bfloat16, tag="aw")
                        nc.scalar.activation(aw[:], r[:],
                                             mybir.ActivationFunctionType.Exp,
                                             bias=nbias, scale=BIG)
                        # scatter the 16 per-128-block transposes into the AWT strip
                        out_3d = awt[:].rearrange("p (b q) -> p b q", b=JC_PER_STRIP)[
                            :, :, ib * IBLK:(ib + 1) * IBLK]
                        nc.sync.dma_start_transpose(out_3d, aw[:])
                    for jc in range(JC_PER_STRIP):
                        g = js * JC_PER_STRIP + jc     # global j-chunk
                        first = (js == 0 and jc == 0)
                        last = (js == NSTRIP - 1 and jc == JC_PER_STRIP - 1)
                        rhs = awt[:, jc * ICHUNK:(jc + 1) * ICHUNK]
                        nc.tensor.matmul(ps_num[:], e4[:, g * D:(g + 1) * D], rhs,
                                         start=first, stop=last)
                        nc.tensor.matmul(ps_z[:], q4[:, g:g + 1], rhs,
                                         start=first, stop=last)
                on = outp.tile([D, ICHUNK], mybir.dt.float32, tag="on")
                nc.vector.tensor_copy(on[:], ps_num[:])
                nc.sync.dma_start(out=numt_d[:, ic * ICHUNK:(ic + 1) * ICHUNK], in_=on[:])
                oz = outp.tile([1, ICHUNK], mybir.dt.float32, tag="oz")
                nc.scalar.copy(oz[:], ps_z[:])
                nc.sync.dma_start(out=z_d[:, ic * ICHUNK:(ic + 1) * ICHUNK], in_=oz[:])

    nc.compile()
    _cache["nc"] = nc
    return nc


def prep_in_maps(adj: np.ndarray, emb: np.ndarray, attn: np.ndarray) -> list:
    emb64 = emb.astype(np.float64)
    s1 = (emb64 @ attn[:D, 0].astype(np.float64)).astype(np.float32)
    s2 = (emb64 @ attn[D:, 0].astype(np.float64)).astype(np.float32)

    q4f = np.exp(s2.astype(np.float64)).astype(np.float32)       # exp(s2)
    e4f = (q4f[:, None] * emb)                                   # (N, D) f32
    # device layouts
    e4_dev = np.ascontiguousarray(
        e4f.reshape(NJC, 128, D).transpose(1, 0, 2).reshape(128, NJC * D)
    ).astype(ml_dtypes.bfloat16)
    q4_dev = np.ascontiguousarray(q4f.reshape(NJC, 128).T).astype(ml_dtypes.bfloat16)

    s2_scaled = (-0.8 / BIG) * s2

    in_maps = []
    for c in range(NCORES):
        rows = slice(c * NI_CORE, (c + 1) * NI_CORE)
        s1c = s1[rows]
        bias_cols = (-0.8 / BIG) * s1c.reshape(NI_CORE // IBLK, IBLK).T  # (128, 8)
        pre = np.empty((128, 9 + N), np.float32)
        pre[:, :8] = bias_cols
        pre[:, 8] = -BIG
        pre[:, 9:] = s2_scaled[None, :]
        in_maps.append({
            "adjs": np.ascontiguousarray(adj[rows]),
            "pre": pre,
            "e4": e4_dev,
            "q4": q4_dev,
        })
    return in_maps


def kernel(adj: np.ndarray, emb: np.ndarray, attn: np.ndarray) -> np.ndarray:
    in_maps = prep_in_maps(adj, emb, attn)
    nc = _build_program()
    res = run_bass_kernel_spmd(nc, in_maps, core_ids=list(range(NCORES)))

    out = np.empty((N, D), np.float32)
    for c, r in enumerate(res.results):
        numt = r["numt"]          # (D, NI_CORE)
        z = r["z"]                # (1, NI_CORE)
        out[c * NI_CORE:(c + 1) * NI_CORE] = (numt / z).T
    return out
